# revision 41
# baseline (speedup 1.0000x reference)
"""Trainium2 Bass kernel for nn_CNFModel: CNF log-density via fixed-step dopri5
with Hutchinson divergence (exact forward-mode JVP).

Contract: kernel(**inputs) takes FULL unsharded inputs (as in setup_inputs())
and returns the FULL [32768, 1] float32 output. Internally shards the batch
across 8 NeuronCores (pure data parallel), runs a Bass/Tile kernel per core,
and gathers.

Per core: 4096 rows as 4 PAIRS of 512-column chunks, feature-major.
v3 design:
 - custom DVE op DTANH_MUL: m = (1-h^2)*u in ONE Vector instruction (replaces
   the square+mul two-op chain that monopolized GpSimd/ScalarE in v1)
 - tangent m-tiles stored fp8e4m3 as [128, 2, NB2] (two contiguous K-blocks);
   u2/u3/psj matmuls run fp8 DoubleRow: K=256 in one pass at 0.5 cyc/col
 - primal in f32r (full-rate PE); tanh merged to [128, 1024] per (layer, mh)
 - divergence accumulated in one PSUM bank (par0 at partition 0, par1 at 64)
"""
import math
import os
from contextlib import ExitStack

import numpy as np

import concourse.bass as bass
import concourse.tile as tile
from concourse import bacc, mybir
from concourse.bass_utils import run_bass_kernel_spmd

import concourse.dve_ops as dve_ops
from concourse.dve_spec import Spec as _DveSpec, Src0, Src1, C0, lower as _dve_lower
from concourse.dve_uop import DveOpSpec as _DveOpSpec


def _register_dtanh_mul():
    """Custom DVE op: out = (c0 - in0^2) * in1 == (1 - h^2) * u for c0=1."""
    name = "DTANH_MUL_ANT"
    if name in dve_ops._SUB_OPCODE_FOR_NAME:
        return next(op for op in dve_ops.OPS if op.name == name)
    spec = _DveSpec(
        body=(C0 - Src0 * Src0) * Src1,
        reference=lambda in0, in1, c0, c1, c2: (
            (c0 - in0.astype(np.float32) ** 2) * in1
        ).astype(np.float32),
    )
    row = max(dve_ops._SUB_OPCODE_FOR_NAME.values()) + 1
    assert row < 0x20
    dve_ops._SUB_OPCODE_FOR_NAME[name] = row
    shas = {
        ver: _DveOpSpec(name=name, opcode=row, uops=_dve_lower(spec, ver=ver),
                        rd1_en=True).sha(ver)
        for ver in ("v3", "v4")
    }
    op = dve_ops.DveOp(name, spec, subdim=False, uops_sha=shas)
    dve_ops.OPS.append(op)
    dve_ops.CUSTOM_DVE_SPECS[name] = spec
    return op


DTANH_MUL = _register_dtanh_mul()

# ---------------------------------------------------------------- problem dims
DIM = 64
HID = 256
BATCH = 32768
N_CORES = 8
B_CORE = BATCH // N_CORES          # 4096
NB = 512                           # per-chunk batch columns
NB2 = 2 * NB                       # pair-merged free size
N_CHUNK = B_CORE // NB             # 8 chunks = 4 pairs
N_STEPS = 4
H = 1.0 / N_STEPS
LOG_2PI = float(np.log(2.0 * np.pi))

_A = [
    [1 / 5],
    [3 / 40, 9 / 40],
    [44 / 45, -56 / 15, 32 / 9],
    [19372 / 6561, -25360 / 2187, 64448 / 6561, -212 / 729],
    [9017 / 3168, -355 / 33, 46732 / 5247, 49 / 176, -5103 / 18656],
]
_B = [35 / 384, 0.0, 500 / 1113, 125 / 192, -2187 / 6784, 11 / 84]

F32 = mybir.dt.float32
F32R = mybir.dt.float32r
BF16 = mybir.dt.bfloat16
FP8 = mybir.dt.float8e4
DR = mybir.MatmulPerfMode.DoubleRow
TANH = mybir.ActivationFunctionType.Tanh
IDENT = mybir.ActivationFunctionType.Identity
MULT = mybir.AluOpType.mult
ADD = mybir.AluOpType.add
SUB = mybir.AluOpType.subtract

_KSLOT = {1: (0, 1), 2: (1, 0), 3: (1, 1), 4: (2, 0), 5: (2, 1), 6: (3, 0)}
_TANGENT = [True, False, True, True, True, True]
# tangent layers using fp8 DoubleRow (m-tile + weight in fp8): subset of
# {"u2", "u3", "psj"}; m0 feeds u2, m1 feeds u3, m2 feeds psj
FP8_LAYERS = set(
    os.environ.get("FP8_LAYERS", "u2,u3,psj").split(",")) - {""}
KILL_TANGENT = os.environ.get("KILL_TANGENT", "0") == "1"
ACC_DVE = os.environ.get("ACC_DVE", "0") == "1"
M0_POOL = os.environ.get("M0_POOL", "0") == "1"
PSA_BUFS = int(os.environ.get("PSA_BUFS", "2"))
PSKC_BUFS = int(os.environ.get("PSKC_BUFS", "1"))


def _combo_specs():
    mats = []
    per_combo = []
    combos = []
    for i, row in enumerate(_A):
        combos.append({j + 1: H * a for j, a in enumerate(row)})
    combos.append({j + 1: H * b for j, b in enumerate(_B) if b != 0.0})
    for cf in combos:
        by_slot = {0: [1.0, 0.0]}
        for j, c in cf.items():
            slot, half = _KSLOT[j]
            by_slot.setdefault(slot, [0.0, 0.0])[half] = c
        spec = []
        for slot in sorted(by_slot):
            cl, cu = by_slot[slot]
            m = np.zeros((128, DIM), np.float32)
            m[0:DIM, 0:DIM] = np.eye(DIM, dtype=np.float32) * cl
            m[DIM:128, 0:DIM] = np.eye(DIM, dtype=np.float32) * cu
            mode = "both" if (cl != 0.0 and cu != 0.0) else (
                "lower" if cu == 0.0 else "upper")
            spec.append((slot, len(mats), mode))
            mats.append(m)
        per_combo.append(spec)
    return np.stack(mats), per_combo


_COMBO_MATS, _COMBO_SPECS = _combo_specs()
N_COMBO = _COMBO_MATS.shape[0]


def _ts(i, n):
    return slice(i * n, (i + 1) * n)


def m128(mh):
    return mh * 128


def _build(n_steps=N_STEPS, n_chunk=N_CHUNK, repeat=1):
    assert n_chunk % 2 == 0
    nc = bacc.Bacc(None, target_bir_lowering=False)

    xt = nc.dram_tensor("xt", [DIM, B_CORE], F32, kind="ExternalInput")
    ept = nc.dram_tensor("ept", [DIM, B_CORE], F32, kind="ExternalInput")
    w1t_d = nc.dram_tensor("w1t", [DIM, HID], F32, kind="ExternalInput")
    w2t_d = nc.dram_tensor("w2t", [128, 2 * HID], F32, kind="ExternalInput")
    w3t_d = nc.dram_tensor("w3t", [128, 2 * HID], F32, kind="ExternalInput")
    w4t_d = nc.dram_tensor("w4t", [128, 2 * DIM], F32, kind="ExternalInput")
    bias_d = nc.dram_tensor("bias", [128, 6], F32, kind="ExternalInput")
    b4_d = nc.dram_tensor("b4c", [DIM, 1], F32, kind="ExternalInput")
    w2dr_d = nc.dram_tensor("w2dr", [128, 2, HID], F32, kind="ExternalInput")
    w3dr_d = nc.dram_tensor("w3dr", [128, 2, HID], F32, kind="ExternalInput")
    w4dr_d = nc.dram_tensor("w4dr", [128, 2, DIM], F32, kind="ExternalInput")
    comb_d = nc.dram_tensor("comb", [128, N_COMBO * DIM], F32, kind="ExternalInput")
    divw_d = nc.dram_tensor("divw", [DIM, 1], F32, kind="ExternalInput")
    ones_d = nc.dram_tensor("onesw", [DIM, 1], F32, kind="ExternalInput")
    out_d = nc.dram_tensor("out", [1, B_CORE], F32, kind="ExternalOutput")

    with tile.TileContext(nc) as tc, ExitStack() as ctx:
        consts = ctx.enter_context(tc.tile_pool(name="consts", bufs=1))
        state = ctx.enter_context(tc.tile_pool(name="state", bufs=1))
        work = ctx.enter_context(tc.tile_pool(name="work", bufs=2))
        pro = ctx.enter_context(tc.tile_pool(name="pro", bufs=1))
        # disjoint per-slot PSUM pools so the two interleaved pairs never
        # serialize on bank rotation: per slot 2 banks psA + 1 bank psKC,
        # plus a shared 2-tag psD (8 banks total)
        psA_ = [ctx.enter_context(tc.tile_pool(name=f"psA{i}", bufs=PSA_BUFS,
                                               space="PSUM")) for i in (0, 1)]
        psKC_ = [ctx.enter_context(tc.tile_pool(name=f"psKC{i}", bufs=PSKC_BUFS,
                                                space="PSUM")) for i in (0, 1)]
        psD = ctx.enter_context(tc.tile_pool(name="psD", bufs=1, space="PSUM"))

        def load_const(dram, shape, tag, dt):
            tmp = pro.tile(shape, F32, tag="ldtmp", name="ldtmp")
            nc.sync.dma_start(out=tmp, in_=dram[...])
            r = consts.tile(shape, dt, tag=tag, name=tag)
            nc.vector.tensor_copy(r, tmp)
            return r

        w1t = load_const(w1t_d, [DIM, HID], "w1t", F32R)
        w2f = load_const(w2dr_d, [128, 2, HID], "w2f", FP8)
        w3f = load_const(w3dr_d, [128, 2, HID], "w3f", FP8)
        w4f = load_const(w4dr_d, [128, 2, DIM], "w4f", FP8)
        w2b = load_const(w2t_d, [128, 2 * HID], "w2b", BF16)
        w3b = load_const(w3t_d, [128, 2 * HID], "w3b", BF16)
        w4b = load_const(w4t_d, [128, 2 * DIM], "w4b", BF16)
        comb = load_const(comb_d, [128, N_COMBO * DIM], "comb", F32R)
        divwb = load_const(divw_d, [DIM, 1], "divw", BF16)
        onesw = load_const(ones_d, [DIM, 1], "onesw", F32R)
        bias = consts.tile([128, 6], F32, tag="bias", name="bias")
        nc.sync.dma_start(out=bias, in_=bias_d[:, :])
        b4 = consts.tile([DIM, 1], F32, tag="b4", name="b4")
        nc.sync.dma_start(out=b4, in_=b4_d[:, :])

        wlf = [w2f, w3f]
        wlb = [w2b, w3b]

        def dtanh_mul(out, h, u):
            # out = (1 - h^2) * u in one DVE instruction
            h_in = h.bitcast(F32) if h.dtype == F32R else h
            nc.vector._custom_dve(DTANH_MUL, out=out, in0=h_in, in1=u, s0=1.0)

        def primal_emit(stage, accs, ksts, pend, sl):
            """Primal pass (generator): yields after each layer so two pair
            streams interleave at layer granularity — engines run their
            queues strictly in order, so fine interleaving is what lets
            pair B fill pair A's dependency stalls."""
            hs = []
            for li in range(3):
                h_pair = [work.tile([128, NB2], BF16, tag=f"h{li}_{mh}_{sl}",
                                    name=f"h{li}_{mh}") for mh in (0, 1)]
                # [128, NB] one-bank PSUM tiles: tanh fires per (mh, par)
                # quarter so the PE->ACT pipeline flows at 1-bank granularity
                for mh in (0, 1):
                    for par in (0, 1):
                        pa = psA_[sl].tile([128, NB], F32, tag="a",
                                           name=f"pa{li}{mh}{par}")
                        if li == 0:
                            nc.tensor.matmul(pa, lhsT=w1t[:, _ts(mh, 128)],
                                             rhs=accs[par], start=True, stop=True)
                        else:
                            w = wlb[li - 1]
                            for kc in (0, 1):
                                nc.tensor.matmul(
                                    pa,
                                    lhsT=w[:, kc * HID + m128(mh): kc * HID + m128(mh + 1)],
                                    rhs=hs[li - 1][kc][:, _ts(par, NB)],
                                    start=(kc == 0), stop=(kc == 1))
                        nc.scalar.activation(h_pair[mh][:, _ts(par, NB)], pa,
                                             TANH,
                                             bias=bias[:, li * 2 + mh: li * 2 + mh + 1])
                next(pend)
                hs.append(h_pair)
                yield
            psks = [psKC_[sl].tile([DIM, NB], F32, tag="kc", name=f"psk{par}")
                    for par in (0, 1)]
            for kc in (0, 1):
                for par in (0, 1):
                    nc.tensor.matmul(psks[par], lhsT=w4b[:, _ts(kc, DIM)],
                                     rhs=hs[2][kc][:, _ts(par, NB)],
                                     start=(kc == 0), stop=(kc == 1))
            slot, half = _KSLOT[stage + 1]
            for par in (0, 1):
                kz_dst = ksts[par][half * DIM:(half + 1) * DIM, _ts(slot, NB)]
                nc.vector.tensor_scalar_add(kz_dst, psks[par], b4[:, 0:1])
            next(pend)
            return hs

        def noop_gen():
            while True:
                yield

        def tangent_pieces(stage, hs, t1, epb, div_ps, sl):
            """Tangent of `stage`, emitted piecewise between the NEXT stage's
            primal layers. m-tiles are [128, 2, NB2] fp8 feeding DoubleRow
            matmuls (K=256 in one pass)."""
            if KILL_TANGENT or not _TANGENT[stage]:
                while True:
                    yield
            u2f = "u2" in FP8_LAYERS and not M0_POOL
            u3f = "u3" in FP8_LAYERS
            pjf = "psj" in FP8_LAYERS
            m0 = work.tile([128, 2, NB2], FP8 if u2f else BF16, tag=f"m0_{sl}",
                           name="m0")
            if M0_POOL:
                sq = work.tile([128, NB2], BF16, tag=f"hsq_{sl}", name="hsq")
            for kc in (0, 1):
                if M0_POOL:
                    nc.gpsimd.tensor_mul(sq, hs[0][kc], hs[0][kc])
                    nc.gpsimd.scalar_tensor_tensor(m0[:, kc:kc + 1, :], sq, 1.0,
                                                   t1[kc], SUB, MULT)
                else:
                    dtanh_mul(m0[:, kc:kc + 1, :], hs[0][kc], t1[kc])
            m_prev = m0
            for li in (1, 2):
                fp8_in = u2f if li == 1 else u3f
                fp8_out = u3f if li == 1 else pjf
                m_next = work.tile([128, 2, NB2], FP8 if fp8_out else BF16,
                                   tag=f"m{li}_{sl}", name=f"m{li}")
                for mh in (0, 1):
                    for par in (0, 1):
                        pu = psA_[sl].tile([128, NB], F32, tag="a",
                                           name=f"pu{mh}{par}")
                        if fp8_in:
                            nc.tensor.matmul(
                                pu,
                                lhsT=wlf[li - 1][:, :, _ts(mh, 128)],
                                rhs=m_prev[:, :, _ts(par, NB)],
                                start=True, stop=True, perf_mode=DR)
                        else:
                            w = wlb[li - 1]
                            for kc in (0, 1):
                                nc.tensor.matmul(
                                    pu,
                                    lhsT=w[:, kc * HID + m128(mh): kc * HID + m128(mh + 1)],
                                    rhs=m_prev[:, kc:kc + 1, _ts(par, NB)],
                                    start=(kc == 0), stop=(kc == 1))
                        dtanh_mul(m_next[:, mh:mh + 1, _ts(par, NB)],
                                  hs[li][mh][:, _ts(par, NB)], pu)
                m_prev = m_next
                yield
            q = work.tile([DIM, NB2], BF16, tag=f"q_{sl}", name="q")
            hb = float(H * _B[stage])
            if M0_POOL:
                hb = -hb
            for par in (0, 1):
                psj = psKC_[sl].tile([DIM, NB], F32, tag="kc", name="psj")
                if pjf:
                    nc.tensor.matmul(psj, lhsT=w4f[:, :, :],
                                     rhs=m_prev[:, :, _ts(par, NB)],
                                     start=True, stop=True, perf_mode=DR)
                else:
                    for kc in (0, 1):
                        nc.tensor.matmul(psj, lhsT=w4b[:, _ts(kc, DIM)],
                                         rhs=m_prev[:, kc:kc + 1, _ts(par, NB)],
                                         start=(kc == 0), stop=(kc == 1))
                nc.vector.scalar_tensor_tensor(q[:, _ts(par, NB)], psj, hb,
                                               epb[:, _ts(par, NB)], MULT, MULT)
            for par in (0, 1):
                nc.tensor.matmul(div_ps[par][0:1, :],
                                 lhsT=divwb[:, 0:1],
                                 rhs=q[:, _ts(par, NB)],
                                 start=(stage == 0), stop=(stage == 5))
            while True:
                yield

        def emit_combo(spec, kst, sl):
            psc = psKC_[sl].tile([DIM, NB], F32, tag="kc", name="psc")
            for idx, (slot, mi, mode) in enumerate(spec):
                if mode == "both":
                    lhsT = comb[:, _ts(mi, DIM)]
                    rhs = kst[:, _ts(slot, NB)]
                elif mode == "lower":
                    lhsT = comb[0:DIM, _ts(mi, DIM)]
                    rhs = kst[0:DIM, _ts(slot, NB)]
                else:
                    lhsT = comb[DIM:128, _ts(mi, DIM)]
                    rhs = kst[DIM:128, _ts(slot, NB)]
                nc.tensor.matmul(psc, lhsT=lhsT, rhs=rhs,
                                 start=(idx == 0), stop=(idx == len(spec) - 1))
            return psc

        # ================================================= pair loop
        def pair_stream(pair, sl):
            """Generator emitting one pair's work, yielding after each dopri5
            stage so two pairs can interleave (fills dependency stalls)."""
            cA, cB = 2 * pair, 2 * pair + 1
            ksts, logps = [], []
            epb = state.tile([DIM, NB2], BF16, tag=f"epb_{sl}", name="epb")
            t1 = [state.tile([128, NB2], BF16, tag=f"t1_0_{sl}", name="t1_0"),
                  state.tile([128, NB2], BF16, tag=f"t1_1_{sl}", name="t1_1")]
            for par, c in ((0, cA), (1, cB)):
                kst = state.tile([128, 4 * NB], F32R, tag=f"kst{par}_{sl}",
                                 name=f"kst{par}")
                xz = pro.tile([DIM, NB], F32, tag="xz", name="xz")
                ep = pro.tile([DIM, NB], F32, tag="ep", name="ep")
                nc.sync.dma_start(out=xz, in_=xt[:, _ts(c, NB)])
                nc.sync.dma_start(out=ep, in_=ept[:, _ts(c, NB)])
                nc.vector.tensor_copy(kst[0:DIM, 0:NB], xz)
                nc.vector.tensor_copy(epb[:, _ts(par, NB)], ep)
                ep_r = pro.tile([DIM, NB], F32R, tag="epr", name="epr")
                nc.vector.tensor_copy(ep_r, ep)
                # T1 = W1 @ eps
                for kc in (0, 1):
                    pa = psKC_[sl].tile([128, NB], F32, tag="kc", name="paT")
                    nc.tensor.matmul(pa, lhsT=w1t[:, _ts(kc, 128)],
                                     rhs=ep_r, start=True, stop=True)
                    nc.vector.tensor_copy(t1[kc][:, _ts(par, NB)], pa)
                logp = work.tile([1, NB], F32, tag=f"logp{par}_{sl}",
                                 name=f"logp{par}")
                nc.vector.memset(logp, 0.0)
                ksts.append(kst)
                logps.append(logp)
                yield
            yield

            for s in range(n_steps):
                # one PSUM bank per slot: par0 div at partition 0, par1 at 64
                div_t = psD.tile([128, NB], F32, tag=f"div{sl}", name="div")
                div_ps = [div_t[0:1, :], div_t[64:65, :]]
                pend = noop_gen()
                for stage in range(6):
                    if stage == 0:
                        accs = [ksts[0][0:DIM, 0:NB], ksts[1][0:DIM, 0:NB]]
                    else:
                        accs = []
                        for par in (0, 1):
                            psc = emit_combo(_COMBO_SPECS[stage - 1], ksts[par], sl)
                            acc = work.tile([DIM, NB], F32R, tag=f"acc{par}_{sl}",
                                            name=f"acc{par}")
                            if ACC_DVE:
                                nc.vector.tensor_copy(acc, psc)
                            else:
                                nc.scalar.activation(acc, psc, IDENT)
                            accs.append(acc)
                    hs = yield from primal_emit(stage, accs, ksts, pend, sl)
                    pend = tangent_pieces(stage, hs, t1, epb, div_ps, sl)
                    yield
                for _ in range(4):
                    next(pend)
                for par in (0, 1):
                    psc = emit_combo(_COMBO_SPECS[5], ksts[par], sl)
                    if ACC_DVE:
                        nc.vector.tensor_copy(ksts[par][0:DIM, 0:NB], psc)
                    else:
                        nc.scalar.activation(ksts[par][0:DIM, 0:NB], psc, IDENT)
                    if not KILL_TANGENT:
                        logp_new = work.tile([1, NB], F32, tag=f"logp{par}_{sl}",
                                             name=f"logp{par}")
                        nc.vector.tensor_add(logp_new, div_ps[par][0:1, :],
                                             logps[par])
                        logps[par] = logp_new
                yield

            for par, c in ((0, cA), (1, cB)):
                yield
                zz = work.tile([DIM, NB], F32R, tag=f"zz_{sl}", name="zz")
                zf = ksts[par][0:DIM, 0:NB].bitcast(F32)
                nc.vector.tensor_mul(zz, zf, zf)
                pslz = psKC_[sl].tile([DIM, NB], F32, tag="kc", name="pslz")
                nc.tensor.matmul(pslz[0:1, 0:NB], lhsT=onesw[:, 0:1], rhs=zz,
                                 start=True, stop=True)
                outt = work.tile([1, NB], F32, tag=f"outt_{sl}", name="outt")
                nc.vector.scalar_tensor_tensor(outt, pslz[0:1, 0:NB],
                                               -0.5 * DIM * LOG_2PI, logps[par],
                                               ADD, SUB)
                nc.sync.dma_start(out=out_d[0:1, _ts(c, NB)], in_=outt)

        def run_pairs():
            # rolling pipeline: two slots, refill a slot with the next pair
            # as soon as its stream ends so boundaries overlap
            n_pairs = n_chunk // 2
            nxt = 0
            streams = [None, None]
            while True:
                alive = False
                for slx in (0, 1):
                    if streams[slx] is None and nxt < n_pairs:
                        streams[slx] = pair_stream(nxt, slx)
                        nxt += 1
                    g = streams[slx]
                    if g is None:
                        continue
                    alive = True
                    try:
                        next(g)
                    except StopIteration:
                        streams[slx] = None
                        if nxt < n_pairs:
                            streams[slx] = pair_stream(nxt, slx)
                            nxt += 1
                if not alive and nxt >= n_pairs:
                    break

        if repeat == 1:
            run_pairs()
        else:
            with tc.For_i(0, repeat, 1):
                run_pairs()

    nc.finalize()
    return nc


def _host_inputs(x, eps, W1, b1, W2, b2, W3, b3, W4, b4):
    x = np.ascontiguousarray(np.asarray(x, dtype=np.float32))
    eps = np.ascontiguousarray(np.asarray(eps, dtype=np.float32))
    W1, W2, W3, W4 = (np.asarray(w, dtype=np.float32) for w in (W1, W2, W3, W4))
    b1, b2, b3, b4 = (np.asarray(b, dtype=np.float32) for b in (b1, b2, b3, b4))

    w1t = np.ascontiguousarray(W1.T)
    w2t = np.ascontiguousarray(
        W2.T.reshape(2, 128, HID).transpose(1, 0, 2).reshape(128, 2 * HID))
    w3t = np.ascontiguousarray(
        W3.T.reshape(2, 128, HID).transpose(1, 0, 2).reshape(128, 2 * HID))
    w4t = np.ascontiguousarray(
        W4.T.reshape(2, 128, DIM).transpose(1, 0, 2).reshape(128, 2 * DIM))
    bias = np.stack([b1[0:128], b1[128:256], b2[0:128], b2[128:256],
                     b3[0:128], b3[128:256]], axis=1).astype(np.float32)
    b4c = b4.reshape(DIM, 1)
    comb = np.ascontiguousarray(
        _COMBO_MATS.transpose(1, 0, 2).reshape(128, N_COMBO * DIM))
    # dtanh_mul gives the true-sign tangent; fold the -div sign of dlogp/dt here
    divw = -np.ones((DIM, 1), np.float32)
    onesw = np.full((DIM, 1), -0.5, np.float32)

    w2dr = np.ascontiguousarray(w2t.reshape(128, 2, HID))
    w3dr = np.ascontiguousarray(w3t.reshape(128, 2, HID))
    w4dr = np.ascontiguousarray(w4t.reshape(128, 2, DIM))
    shared = dict(w1t=w1t, w2t=w2t, w3t=w3t, w4t=w4t, bias=bias, b4c=b4c,
                  w2dr=w2dr, w3dr=w3dr, w4dr=w4dr,
                  comb=comb, divw=divw, onesw=onesw)
    in_maps = []
    for core in range(N_CORES):
        rows = slice(core * B_CORE, (core + 1) * B_CORE)
        m = dict(shared)
        m["xt"] = np.ascontiguousarray(x[rows].T)
        m["ept"] = np.ascontiguousarray(eps[rows].T)
        in_maps.append(m)
    return in_maps


_NC_CACHE = {}


def _get_nc():
    if "full" not in _NC_CACHE:
        _NC_CACHE["full"] = _build()
    return _NC_CACHE["full"]


def _run(in_maps, **kw):
    nc = _get_nc()
    return run_bass_kernel_spmd(nc, in_maps, core_ids=list(range(N_CORES)), **kw)


def kernel(x, eps, W1, b1, W2, b2, W3, b3, W4, b4):
    in_maps = _host_inputs(x, eps, W1, b1, W2, b2, W3, b3, W4, b4)
    res = _run(in_maps)
    outs = [res.results[c]["out"].reshape(B_CORE) for c in range(N_CORES)]
    return np.concatenate(outs).reshape(BATCH, 1).astype(np.float32)


def kernel_traced(x, eps, W1, b1, W2, b2, W3, b3, W4, b4):
    in_maps = _host_inputs(x, eps, W1, b1, W2, b2, W3, b3, W4, b4)
    res = _run(in_maps, trace=True)
    outs = [res.results[c]["out"].reshape(B_CORE) for c in range(N_CORES)]
    return np.concatenate(outs).reshape(BATCH, 1).astype(np.float32), res


# revision 44
# speedup vs baseline: 1.1721x; 1.1721x over previous
"""Trainium2 Bass kernel for nn_CNFModel: CNF log-density via fixed-step dopri5
with Hutchinson divergence (exact forward-mode JVP).

Contract: kernel(**inputs) takes FULL unsharded inputs (as in setup_inputs())
and returns the FULL [32768, 1] float32 output. Internally shards the batch
across 8 NeuronCores (pure data parallel), runs a Bass/Tile kernel per core,
and gathers.

Per core: 4096 rows as 4 PAIRS of 512-column chunks, feature-major.
v3 design:
 - custom DVE op DTANH_MUL: m = (1-h^2)*u in ONE Vector instruction (replaces
   the square+mul two-op chain that monopolized GpSimd/ScalarE in v1)
 - tangent m-tiles stored fp8e4m3 as [128, 2, NB2] (two contiguous K-blocks);
   u2/u3/psj matmuls run fp8 DoubleRow: K=256 in one pass at 0.5 cyc/col
 - primal in f32r (full-rate PE); tanh merged to [128, 1024] per (layer, mh)
 - divergence accumulated in one PSUM bank (par0 at partition 0, par1 at 64)
"""
import math
import os
from contextlib import ExitStack

import numpy as np

import concourse.bass as bass
import concourse.tile as tile
from concourse import bacc, mybir
from concourse.bass_utils import run_bass_kernel_spmd

import concourse.dve_ops as dve_ops
from concourse.dve_spec import Spec as _DveSpec, Src0, Src1, C0, lower as _dve_lower
from concourse.dve_uop import DveOpSpec as _DveOpSpec


def _register_dtanh_mul():
    """Custom DVE op: out = (c0 - in0^2) * in1 == (1 - h^2) * u for c0=1."""
    name = "DTANH_MUL_ANT"
    if name in dve_ops._SUB_OPCODE_FOR_NAME:
        return next(op for op in dve_ops.OPS if op.name == name)
    spec = _DveSpec(
        body=(C0 - Src0 * Src0) * Src1,
        reference=lambda in0, in1, c0, c1, c2: (
            (c0 - in0.astype(np.float32) ** 2) * in1
        ).astype(np.float32),
    )
    row = max(dve_ops._SUB_OPCODE_FOR_NAME.values()) + 1
    assert row < 0x20
    dve_ops._SUB_OPCODE_FOR_NAME[name] = row
    shas = {
        ver: _DveOpSpec(name=name, opcode=row, uops=_dve_lower(spec, ver=ver),
                        rd1_en=True).sha(ver)
        for ver in ("v3", "v4")
    }
    op = dve_ops.DveOp(name, spec, subdim=False, uops_sha=shas)
    dve_ops.OPS.append(op)
    dve_ops.CUSTOM_DVE_SPECS[name] = spec
    return op


DTANH_MUL = _register_dtanh_mul()

# ---------------------------------------------------------------- problem dims
DIM = 64
HID = 256
BATCH = 32768
N_CORES = 8
B_CORE = BATCH // N_CORES          # 4096
NB = 512                           # per-chunk batch columns
NB2 = 2 * NB                       # pair-merged free size
N_CHUNK = B_CORE // NB             # 8 chunks = 4 pairs
N_STEPS = 4
H = 1.0 / N_STEPS
LOG_2PI = float(np.log(2.0 * np.pi))

_A = [
    [1 / 5],
    [3 / 40, 9 / 40],
    [44 / 45, -56 / 15, 32 / 9],
    [19372 / 6561, -25360 / 2187, 64448 / 6561, -212 / 729],
    [9017 / 3168, -355 / 33, 46732 / 5247, 49 / 176, -5103 / 18656],
]
_B = [35 / 384, 0.0, 500 / 1113, 125 / 192, -2187 / 6784, 11 / 84]

F32 = mybir.dt.float32
F32R = mybir.dt.float32r
BF16 = mybir.dt.bfloat16
FP8 = mybir.dt.float8e4
DR = mybir.MatmulPerfMode.DoubleRow
TANH = mybir.ActivationFunctionType.Tanh
IDENT = mybir.ActivationFunctionType.Identity
MULT = mybir.AluOpType.mult
ADD = mybir.AluOpType.add
SUB = mybir.AluOpType.subtract

_KSLOT = {1: (0, 1), 2: (1, 0), 3: (1, 1), 4: (2, 0), 5: (2, 1), 6: (3, 0)}
_TANGENT = [True, False, True, True, True, True]
# tangent layers using fp8 DoubleRow (m-tile + weight in fp8): subset of
# {"u2", "u3", "psj"}; m0 feeds u2, m1 feeds u3, m2 feeds psj
FP8_LAYERS = set(
    os.environ.get("FP8_LAYERS", "u2,u3,psj").split(",")) - {""}
KILL_TANGENT = os.environ.get("KILL_TANGENT", "0") == "1"
ACC_DVE = os.environ.get("ACC_DVE", "0") == "1"
DIV_DVE = os.environ.get("DIV_DVE", "1") == "1"
FINE_YIELD = os.environ.get("FINE_YIELD", "1") == "1"
M0_POOL = os.environ.get("M0_POOL", "0") == "1"
PSA_BUFS = int(os.environ.get("PSA_BUFS", "2"))
PSKC_BUFS = int(os.environ.get("PSKC_BUFS", "2"))


def _combo_specs():
    mats = []
    per_combo = []
    combos = []
    for i, row in enumerate(_A):
        combos.append({j + 1: H * a for j, a in enumerate(row)})
    combos.append({j + 1: H * b for j, b in enumerate(_B) if b != 0.0})
    for cf in combos:
        by_slot = {0: [1.0, 0.0]}
        for j, c in cf.items():
            slot, half = _KSLOT[j]
            by_slot.setdefault(slot, [0.0, 0.0])[half] = c
        spec = []
        for slot in sorted(by_slot):
            cl, cu = by_slot[slot]
            m = np.zeros((128, DIM), np.float32)
            m[0:DIM, 0:DIM] = np.eye(DIM, dtype=np.float32) * cl
            m[DIM:128, 0:DIM] = np.eye(DIM, dtype=np.float32) * cu
            mode = "both" if (cl != 0.0 and cu != 0.0) else (
                "lower" if cu == 0.0 else "upper")
            spec.append((slot, len(mats), mode))
            mats.append(m)
        per_combo.append(spec)
    return np.stack(mats), per_combo


_COMBO_MATS, _COMBO_SPECS = _combo_specs()
N_COMBO = _COMBO_MATS.shape[0]


def _ts(i, n):
    return slice(i * n, (i + 1) * n)


def m128(mh):
    return mh * 128


def _build(n_steps=N_STEPS, n_chunk=N_CHUNK, repeat=1):
    assert n_chunk % 2 == 0
    nc = bacc.Bacc(None, target_bir_lowering=False)

    xt = nc.dram_tensor("xt", [DIM, B_CORE], F32, kind="ExternalInput")
    ept = nc.dram_tensor("ept", [DIM, B_CORE], F32, kind="ExternalInput")
    w1t_d = nc.dram_tensor("w1t", [DIM, HID], F32, kind="ExternalInput")
    w2t_d = nc.dram_tensor("w2t", [128, 2 * HID], F32, kind="ExternalInput")
    w3t_d = nc.dram_tensor("w3t", [128, 2 * HID], F32, kind="ExternalInput")
    w4t_d = nc.dram_tensor("w4t", [128, 2 * DIM], F32, kind="ExternalInput")
    bias_d = nc.dram_tensor("bias", [128, 6], F32, kind="ExternalInput")
    b4_d = nc.dram_tensor("b4c", [DIM, 1], F32, kind="ExternalInput")
    w2dr_d = nc.dram_tensor("w2dr", [128, 2, HID], F32, kind="ExternalInput")
    w3dr_d = nc.dram_tensor("w3dr", [128, 2, HID], F32, kind="ExternalInput")
    w4dr_d = nc.dram_tensor("w4dr", [128, 2, DIM], F32, kind="ExternalInput")
    comb_d = nc.dram_tensor("comb", [128, N_COMBO * DIM], F32, kind="ExternalInput")
    divw_d = nc.dram_tensor("divw", [DIM, 1], F32, kind="ExternalInput")
    ones_d = nc.dram_tensor("onesw", [DIM, 1], F32, kind="ExternalInput")
    out_d = nc.dram_tensor("out", [1, B_CORE], F32, kind="ExternalOutput")

    with tile.TileContext(nc) as tc, ExitStack() as ctx:
        consts = ctx.enter_context(tc.tile_pool(name="consts", bufs=1))
        state = ctx.enter_context(tc.tile_pool(name="state", bufs=1))
        work = ctx.enter_context(tc.tile_pool(name="work", bufs=2))
        pro = ctx.enter_context(tc.tile_pool(name="pro", bufs=1))
        # disjoint per-slot PSUM pools so the two interleaved pairs never
        # serialize on bank rotation: per slot 2 banks psA + 1 bank psKC,
        # plus a shared 2-tag psD (8 banks total)
        psA_ = [ctx.enter_context(tc.tile_pool(name=f"psA{i}", bufs=PSA_BUFS,
                                               space="PSUM")) for i in (0, 1)]
        psKC_ = [ctx.enter_context(tc.tile_pool(name=f"psKC{i}", bufs=PSKC_BUFS,
                                                space="PSUM")) for i in (0, 1)]
        psD = None if DIV_DVE else ctx.enter_context(
            tc.tile_pool(name="psD", bufs=1, space="PSUM"))

        def load_const(dram, shape, tag, dt):
            tmp = pro.tile(shape, F32, tag="ldtmp", name="ldtmp")
            nc.sync.dma_start(out=tmp, in_=dram[...])
            r = consts.tile(shape, dt, tag=tag, name=tag)
            nc.vector.tensor_copy(r, tmp)
            return r

        w1t = load_const(w1t_d, [DIM, HID], "w1t", F32R)
        w2f = load_const(w2dr_d, [128, 2, HID], "w2f", FP8)
        w3f = load_const(w3dr_d, [128, 2, HID], "w3f", FP8)
        w4f = load_const(w4dr_d, [128, 2, DIM], "w4f", FP8)
        w2b = load_const(w2t_d, [128, 2 * HID], "w2b", BF16)
        w3b = load_const(w3t_d, [128, 2 * HID], "w3b", BF16)
        w4b = load_const(w4t_d, [128, 2 * DIM], "w4b", BF16)
        comb = load_const(comb_d, [128, N_COMBO * DIM], "comb", F32R)
        divwb = load_const(divw_d, [DIM, 1], "divw", BF16)
        onesw = load_const(ones_d, [DIM, 1], "onesw", F32R)
        bias = consts.tile([128, 6], F32, tag="bias", name="bias")
        nc.sync.dma_start(out=bias, in_=bias_d[:, :])
        b4 = consts.tile([DIM, 1], F32, tag="b4", name="b4")
        nc.sync.dma_start(out=b4, in_=b4_d[:, :])

        wlf = [w2f, w3f]
        wlb = [w2b, w3b]

        def dtanh_mul(out, h, u):
            # out = (1 - h^2) * u in one DVE instruction
            h_in = h.bitcast(F32) if h.dtype == F32R else h
            nc.vector._custom_dve(DTANH_MUL, out=out, in0=h_in, in1=u, s0=1.0)

        def primal_emit(stage, accs, ksts, pend, sl):
            """Primal pass (generator): yields after each layer so two pair
            streams interleave at layer granularity — engines run their
            queues strictly in order, so fine interleaving is what lets
            pair B fill pair A's dependency stalls."""
            hs = []
            for li in range(3):
                h_pair = [work.tile([128, NB2], BF16, tag=f"h{li}_{mh}_{sl}",
                                    name=f"h{li}_{mh}") for mh in (0, 1)]
                # [128, NB] one-bank PSUM tiles: tanh fires per (mh, par)
                # quarter so the PE->ACT pipeline flows at 1-bank granularity
                for mh in (0, 1):
                    for par in (0, 1):
                        pa = psA_[sl].tile([128, NB], F32, tag="a",
                                           name=f"pa{li}{mh}{par}")
                        if li == 0:
                            nc.tensor.matmul(pa, lhsT=w1t[:, _ts(mh, 128)],
                                             rhs=accs[par], start=True, stop=True)
                        else:
                            w = wlb[li - 1]
                            for kc in (0, 1):
                                nc.tensor.matmul(
                                    pa,
                                    lhsT=w[:, kc * HID + m128(mh): kc * HID + m128(mh + 1)],
                                    rhs=hs[li - 1][kc][:, _ts(par, NB)],
                                    start=(kc == 0), stop=(kc == 1))
                        nc.scalar.activation(h_pair[mh][:, _ts(par, NB)], pa,
                                             TANH,
                                             bias=bias[:, li * 2 + mh: li * 2 + mh + 1])
                        if FINE_YIELD:
                            yield
                next(pend)
                hs.append(h_pair)
                yield
            psks = [psKC_[sl].tile([DIM, NB], F32, tag="kc", name=f"psk{par}")
                    for par in (0, 1)]
            for kc in (0, 1):
                for par in (0, 1):
                    nc.tensor.matmul(psks[par], lhsT=w4b[:, _ts(kc, DIM)],
                                     rhs=hs[2][kc][:, _ts(par, NB)],
                                     start=(kc == 0), stop=(kc == 1))
            slot, half = _KSLOT[stage + 1]
            for par in (0, 1):
                kz_dst = ksts[par][half * DIM:(half + 1) * DIM, _ts(slot, NB)]
                nc.vector.tensor_scalar_add(kz_dst, psks[par], b4[:, 0:1])
            next(pend)
            return hs

        def noop_gen():
            while True:
                yield

        def tangent_pieces(stage, hs, t1, epb, div_ps, sl, logps):
            """Tangent of `stage`, emitted piecewise between the NEXT stage's
            primal layers. m-tiles are [128, 2, NB2] fp8 feeding DoubleRow
            matmuls (K=256 in one pass)."""
            if KILL_TANGENT or not _TANGENT[stage]:
                while True:
                    yield
            u2f = "u2" in FP8_LAYERS and not M0_POOL
            u3f = "u3" in FP8_LAYERS
            pjf = "psj" in FP8_LAYERS
            m0 = work.tile([128, 2, NB2], FP8 if u2f else BF16, tag=f"m0_{sl}",
                           name="m0")
            if M0_POOL:
                sq = work.tile([128, NB2], BF16, tag=f"hsq_{sl}", name="hsq")
            for kc in (0, 1):
                if M0_POOL:
                    nc.gpsimd.tensor_mul(sq, hs[0][kc], hs[0][kc])
                    nc.gpsimd.scalar_tensor_tensor(m0[:, kc:kc + 1, :], sq, 1.0,
                                                   t1[kc], SUB, MULT)
                else:
                    dtanh_mul(m0[:, kc:kc + 1, :], hs[0][kc], t1[kc])
            m_prev = m0
            for li in (1, 2):
                fp8_in = u2f if li == 1 else u3f
                fp8_out = u3f if li == 1 else pjf
                m_next = work.tile([128, 2, NB2], FP8 if fp8_out else BF16,
                                   tag=f"m{li}_{sl}", name=f"m{li}")
                for mh in (0, 1):
                    for par in (0, 1):
                        pu = psA_[sl].tile([128, NB], F32, tag="a",
                                           name=f"pu{mh}{par}")
                        if fp8_in:
                            nc.tensor.matmul(
                                pu,
                                lhsT=wlf[li - 1][:, :, _ts(mh, 128)],
                                rhs=m_prev[:, :, _ts(par, NB)],
                                start=True, stop=True, perf_mode=DR)
                        else:
                            w = wlb[li - 1]
                            for kc in (0, 1):
                                nc.tensor.matmul(
                                    pu,
                                    lhsT=w[:, kc * HID + m128(mh): kc * HID + m128(mh + 1)],
                                    rhs=m_prev[:, kc:kc + 1, _ts(par, NB)],
                                    start=(kc == 0), stop=(kc == 1))
                        dtanh_mul(m_next[:, mh:mh + 1, _ts(par, NB)],
                                  hs[li][mh][:, _ts(par, NB)], pu)
                m_prev = m_next
                yield
            q = work.tile([DIM, NB2], BF16, tag=f"q_{sl}", name="q")
            hb = float(H * _B[stage])
            if M0_POOL:
                hb = -hb
            for par in (0, 1):
                psj = psKC_[sl].tile([DIM, NB], F32, tag="kc", name="psj")
                if pjf:
                    nc.tensor.matmul(psj, lhsT=w4f[:, :, :],
                                     rhs=m_prev[:, :, _ts(par, NB)],
                                     start=True, stop=True, perf_mode=DR)
                else:
                    for kc in (0, 1):
                        nc.tensor.matmul(psj, lhsT=w4b[:, _ts(kc, DIM)],
                                         rhs=m_prev[:, kc:kc + 1, _ts(par, NB)],
                                         start=(kc == 0), stop=(kc == 1))
                nc.vector.scalar_tensor_tensor(q[:, _ts(par, NB)], psj, hb,
                                               epb[:, _ts(par, NB)], MULT, MULT)
            if DIV_DVE:
                divt = psKC_[sl].tile([128, NB], F32, tag="kc", name="divt")
                for par in (0, 1):
                    nc.tensor.matmul(divt[par * 64:par * 64 + 1, :],
                                     lhsT=divwb[:, 0:1],
                                     rhs=q[:, _ts(par, NB)],
                                     start=True, stop=True)
                for par in (0, 1):
                    logp_new = work.tile([1, NB], F32, tag=f"logp{par}_{sl}",
                                         name=f"logp{par}")
                    nc.vector.tensor_add(logp_new,
                                         divt[par * 64:par * 64 + 1, :],
                                         logps[par])
                    logps[par] = logp_new
            else:
                for par in (0, 1):
                    nc.tensor.matmul(div_ps[par][0:1, :],
                                     lhsT=divwb[:, 0:1],
                                     rhs=q[:, _ts(par, NB)],
                                     start=(stage == 0), stop=(stage == 5))
            while True:
                yield

        def emit_combo(spec, kst, sl):
            psc = psKC_[sl].tile([DIM, NB], F32, tag="kc", name="psc")
            for idx, (slot, mi, mode) in enumerate(spec):
                if mode == "both":
                    lhsT = comb[:, _ts(mi, DIM)]
                    rhs = kst[:, _ts(slot, NB)]
                elif mode == "lower":
                    lhsT = comb[0:DIM, _ts(mi, DIM)]
                    rhs = kst[0:DIM, _ts(slot, NB)]
                else:
                    lhsT = comb[DIM:128, _ts(mi, DIM)]
                    rhs = kst[DIM:128, _ts(slot, NB)]
                nc.tensor.matmul(psc, lhsT=lhsT, rhs=rhs,
                                 start=(idx == 0), stop=(idx == len(spec) - 1))
            return psc

        # ================================================= pair loop
        def pair_stream(pair, sl):
            """Generator emitting one pair's work, yielding after each dopri5
            stage so two pairs can interleave (fills dependency stalls)."""
            cA, cB = 2 * pair, 2 * pair + 1
            ksts, logps = [], []
            epb = state.tile([DIM, NB2], BF16, tag=f"epb_{sl}", name="epb")
            t1 = [state.tile([128, NB2], BF16, tag=f"t1_0_{sl}", name="t1_0"),
                  state.tile([128, NB2], BF16, tag=f"t1_1_{sl}", name="t1_1")]
            for par, c in ((0, cA), (1, cB)):
                kst = state.tile([128, 4 * NB], F32R, tag=f"kst{par}_{sl}",
                                 name=f"kst{par}")
                xz = pro.tile([DIM, NB], F32, tag="xz", name="xz")
                ep = pro.tile([DIM, NB], F32, tag="ep", name="ep")
                nc.sync.dma_start(out=xz, in_=xt[:, _ts(c, NB)])
                nc.sync.dma_start(out=ep, in_=ept[:, _ts(c, NB)])
                nc.vector.tensor_copy(kst[0:DIM, 0:NB], xz)
                nc.vector.tensor_copy(epb[:, _ts(par, NB)], ep)
                ep_r = pro.tile([DIM, NB], F32R, tag="epr", name="epr")
                nc.vector.tensor_copy(ep_r, ep)
                # T1 = W1 @ eps
                for kc in (0, 1):
                    pa = psKC_[sl].tile([128, NB], F32, tag="kc", name="paT")
                    nc.tensor.matmul(pa, lhsT=w1t[:, _ts(kc, 128)],
                                     rhs=ep_r, start=True, stop=True)
                    nc.vector.tensor_copy(t1[kc][:, _ts(par, NB)], pa)
                logp = work.tile([1, NB], F32, tag=f"logp{par}_{sl}",
                                 name=f"logp{par}")
                nc.vector.memset(logp, 0.0)
                ksts.append(kst)
                logps.append(logp)
                yield
            yield

            for s in range(n_steps):
                if DIV_DVE:
                    div_ps = None
                else:
                    div_t = psD.tile([128, NB], F32, tag=f"div{sl}", name="div")
                    div_ps = [div_t[0:1, :], div_t[64:65, :]]
                pend = noop_gen()
                for stage in range(6):
                    if stage == 0:
                        accs = [ksts[0][0:DIM, 0:NB], ksts[1][0:DIM, 0:NB]]
                    else:
                        accs = []
                        for par in (0, 1):
                            psc = emit_combo(_COMBO_SPECS[stage - 1], ksts[par], sl)
                            acc = work.tile([DIM, NB], F32R, tag=f"acc{par}_{sl}",
                                            name=f"acc{par}")
                            if ACC_DVE:
                                nc.vector.tensor_copy(acc, psc)
                            else:
                                nc.scalar.activation(acc, psc, IDENT)
                            accs.append(acc)
                            if FINE_YIELD:
                                yield
                    hs = yield from primal_emit(stage, accs, ksts, pend, sl)
                    pend = tangent_pieces(stage, hs, t1, epb, div_ps, sl, logps)
                    yield
                for _ in range(4):
                    next(pend)
                for par in (0, 1):
                    psc = emit_combo(_COMBO_SPECS[5], ksts[par], sl)
                    if ACC_DVE:
                        nc.vector.tensor_copy(ksts[par][0:DIM, 0:NB], psc)
                    else:
                        nc.scalar.activation(ksts[par][0:DIM, 0:NB], psc, IDENT)
                    if not KILL_TANGENT and not DIV_DVE:
                        logp_new = work.tile([1, NB], F32, tag=f"logp{par}_{sl}",
                                             name=f"logp{par}")
                        nc.vector.tensor_add(logp_new, div_ps[par][0:1, :],
                                             logps[par])
                        logps[par] = logp_new
                yield

            for par, c in ((0, cA), (1, cB)):
                yield
                zz = work.tile([DIM, NB], F32R, tag=f"zz_{sl}", name="zz")
                zf = ksts[par][0:DIM, 0:NB].bitcast(F32)
                nc.vector.tensor_mul(zz, zf, zf)
                pslz = psKC_[sl].tile([DIM, NB], F32, tag="kc", name="pslz")
                nc.tensor.matmul(pslz[0:1, 0:NB], lhsT=onesw[:, 0:1], rhs=zz,
                                 start=True, stop=True)
                outt = work.tile([1, NB], F32, tag=f"outt_{sl}", name="outt")
                nc.vector.scalar_tensor_tensor(outt, pslz[0:1, 0:NB],
                                               -0.5 * DIM * LOG_2PI, logps[par],
                                               ADD, SUB)
                nc.sync.dma_start(out=out_d[0:1, _ts(c, NB)], in_=outt)

        def run_pairs():
            # rolling pipeline: two slots, refill a slot with the next pair
            # as soon as its stream ends so boundaries overlap
            n_pairs = n_chunk // 2
            nxt = 0
            streams = [None, None]
            while True:
                alive = False
                for slx in (0, 1):
                    if streams[slx] is None and nxt < n_pairs:
                        streams[slx] = pair_stream(nxt, slx)
                        nxt += 1
                    g = streams[slx]
                    if g is None:
                        continue
                    alive = True
                    try:
                        next(g)
                    except StopIteration:
                        streams[slx] = None
                        if nxt < n_pairs:
                            streams[slx] = pair_stream(nxt, slx)
                            nxt += 1
                if not alive and nxt >= n_pairs:
                    break

        if repeat == 1:
            run_pairs()
        else:
            with tc.For_i(0, repeat, 1):
                run_pairs()

    nc.finalize()
    return nc


def _host_inputs(x, eps, W1, b1, W2, b2, W3, b3, W4, b4):
    x = np.ascontiguousarray(np.asarray(x, dtype=np.float32))
    eps = np.ascontiguousarray(np.asarray(eps, dtype=np.float32))
    W1, W2, W3, W4 = (np.asarray(w, dtype=np.float32) for w in (W1, W2, W3, W4))
    b1, b2, b3, b4 = (np.asarray(b, dtype=np.float32) for b in (b1, b2, b3, b4))

    w1t = np.ascontiguousarray(W1.T)
    w2t = np.ascontiguousarray(
        W2.T.reshape(2, 128, HID).transpose(1, 0, 2).reshape(128, 2 * HID))
    w3t = np.ascontiguousarray(
        W3.T.reshape(2, 128, HID).transpose(1, 0, 2).reshape(128, 2 * HID))
    w4t = np.ascontiguousarray(
        W4.T.reshape(2, 128, DIM).transpose(1, 0, 2).reshape(128, 2 * DIM))
    bias = np.stack([b1[0:128], b1[128:256], b2[0:128], b2[128:256],
                     b3[0:128], b3[128:256]], axis=1).astype(np.float32)
    b4c = b4.reshape(DIM, 1)
    comb = np.ascontiguousarray(
        _COMBO_MATS.transpose(1, 0, 2).reshape(128, N_COMBO * DIM))
    # dtanh_mul gives the true-sign tangent; fold the -div sign of dlogp/dt here
    divw = -np.ones((DIM, 1), np.float32)
    onesw = np.full((DIM, 1), -0.5, np.float32)

    w2dr = np.ascontiguousarray(w2t.reshape(128, 2, HID))
    w3dr = np.ascontiguousarray(w3t.reshape(128, 2, HID))
    w4dr = np.ascontiguousarray(w4t.reshape(128, 2, DIM))
    shared = dict(w1t=w1t, w2t=w2t, w3t=w3t, w4t=w4t, bias=bias, b4c=b4c,
                  w2dr=w2dr, w3dr=w3dr, w4dr=w4dr,
                  comb=comb, divw=divw, onesw=onesw)
    in_maps = []
    for core in range(N_CORES):
        rows = slice(core * B_CORE, (core + 1) * B_CORE)
        m = dict(shared)
        m["xt"] = np.ascontiguousarray(x[rows].T)
        m["ept"] = np.ascontiguousarray(eps[rows].T)
        in_maps.append(m)
    return in_maps


_NC_CACHE = {}


def _get_nc():
    if "full" not in _NC_CACHE:
        _NC_CACHE["full"] = _build()
    return _NC_CACHE["full"]


def _run(in_maps, **kw):
    nc = _get_nc()
    return run_bass_kernel_spmd(nc, in_maps, core_ids=list(range(N_CORES)), **kw)


def kernel(x, eps, W1, b1, W2, b2, W3, b3, W4, b4):
    in_maps = _host_inputs(x, eps, W1, b1, W2, b2, W3, b3, W4, b4)
    res = _run(in_maps)
    outs = [res.results[c]["out"].reshape(B_CORE) for c in range(N_CORES)]
    return np.concatenate(outs).reshape(BATCH, 1).astype(np.float32)


def kernel_traced(x, eps, W1, b1, W2, b2, W3, b3, W4, b4):
    in_maps = _host_inputs(x, eps, W1, b1, W2, b2, W3, b3, W4, b4)
    res = _run(in_maps, trace=True)
    outs = [res.results[c]["out"].reshape(B_CORE) for c in range(N_CORES)]
    return np.concatenate(outs).reshape(BATCH, 1).astype(np.float32), res
